# revision 56
# baseline (speedup 1.0000x reference)
import sys
sys.path.insert(0, "/opt/trn_rl_repo")
import numpy as np
import concourse.bass as bass
import concourse.bacc as bacc
import concourse.tile as tile
from concourse import mybir
from concourse.bass_utils import run_bass_kernel_spmd
from concourse import bass_isa

F32 = mybir.dt.float32
F32R = mybir.dt.float32r
BF16 = mybir.dt.bfloat16
F8E4 = mybir.dt.float8e4
AF = mybir.ActivationFunctionType
OP = mybir.AluOpType
DR = mybir.MatmulPerfMode.DoubleRow

USE_DR = False          # fp8 DoubleRow for the relpos-add matmul
N, CIO, L = 16, 512, 384
DIMHEAD, NUMHEAD, MAXEMBED, DIMGROUP = 64, 8, 384, 8
CHID = DIMHEAD * NUMHEAD
NCORES = 8
BPC = N // NCORES      # batches per core
KC = CIO // 128        # contraction chunks
OC = CHID // 128       # output-channel chunks
JC = L // 128          # key-position chunks

# constant-blob column layout (128-partition blob, fp32r-typed)
CB_VRES = 0            # [128,4]
CB_QKO = 4             # [128,4]
CB_QKP = 8             # [128,4]
CB_DB = 12             # [128,4] dense bias
CB_ONES = 16           # [128,64] ones (vT one-columns + rb broadcast lhsT)
CB_COLS = 80
# 8-partition blob
B8_I8 = 0              # [8,8]
B8_GB = 8              # [8,1] gate bias
B8_COLS = 9


def build_nc(iters=1):
    nc = bacc.Bacc("TRN2", target_bir_lowering=False, debug=False)

    def inp(name, shape, dt):
        return nc.dram_tensor(name, shape, dt, kind="ExternalInput").ap()

    x = inp("x", [BPC, CIO, L], BF16)
    xorg = inp("xorg", [BPC, CIO, L], BF16)
    abspos = inp("abspos", [BPC, CIO, L], BF16)
    mblob = inp("mblob", [BPC, 128, 4], F32)     # [maskT(3) | norm(1)]
    if USE_DR:
        rpt8in = inp("rpt8in", [64, 2 * JC * L], F8E4)
        id8in = inp("id8in", [64, 2 * 128], F8E4)
    else:
        rptin = inp("rptin", [128, JC * L], BF16)
        i128b = inp("i128b", [128, 128], BF16)
    gate_wT = inp("gate_wT", [CIO, NUMHEAD], BF16)
    q_wT = inp("q_wT", [CIO, CHID], F32R)
    k_wT = inp("k_wT", [CIO, CHID], F32R)
    v_wT = inp("v_wT", [CIO, CHID], F32R)
    dense_wT = inp("dense_wT", [CHID, CIO], F32R)
    cblob = inp("cblob", [128, CB_COLS], F32R)
    blob8 = inp("blob8", [NUMHEAD, B8_COLS], F32R)
    out = nc.dram_tensor("out", [BPC, CIO, L], F32, kind="ExternalOutput").ap()

    def chunked_src(t, b):
        # [CIO, L] DRAM slice viewed as [p(128), kc(4), l(384)]
        return bass.AP(tensor=t.tensor, offset=b * CIO * L,
                       ap=[[L, 128], [128 * L, KC], [1, L]])

    def wsrc(t):
        # [CIO, CHID] viewed as [p(128), kc(4), c(CHID)]
        return bass.AP(tensor=t.tensor, offset=0,
                       ap=[[CHID, 128], [128 * CHID, KC], [1, CHID]])

    with tile.TileContext(nc) as tc:
        with tc.tile_pool(name="wts", bufs=1) as wp, \
             tc.tile_pool(name="a2", bufs=2) as a2, \
             tc.tile_pool(name="a1", bufs=1) as a1, \
             tc.tile_pool(name="pp", bufs=6) as ppool, \
             tc.tile_pool(name="ps_big", bufs=3, space="PSUM") as psb, \
             tc.tile_pool(name="ps_s", bufs=3, space="PSUM") as pss, \
             tc.tile_pool(name="ps_o", bufs=2, space="PSUM") as pso:

            first = [True]
            weights = {}

            for it in range(iters):
                binp = []

                def emit_inp(b, ab_engine=None):
                    txa = a1.tile([128, KC * L], BF16, tag=f"xa{b}", name=f"xa{b}")
                    nc.sync.dma_start(out=txa, in_=chunked_src(x, b))
                    txo = a1.tile([128, KC * L], BF16, tag=f"xo{b}", name=f"xo{b}")
                    nc.sync.dma_start(out=txo, in_=chunked_src(xorg, b))
                    tab = a1.tile([128, KC * L], BF16, tag=f"ab{b}", name=f"ab{b}")
                    (ab_engine or nc.gpsimd).dma_start(out=tab,
                                                       in_=chunked_src(abspos, b))
                    binp.append((txa, txo, tab))

                def wdma(nm, shape, dt, srcap):
                    t = wp.tile([128, shape], dt, tag=nm, name=nm)
                    nc.scalar.dma_start(out=t, in_=srcap)
                    weights[nm] = t
                    return t

                if first[0]:
                    w = weights
                    emit_inp(0)
                    t = wp.tile([NUMHEAD, B8_COLS], F32R, tag="b8", name="b8")
                    nc.scalar.dma_start(out=t, in_=blob8)
                    w["b8"] = t
                    wdma("wg", KC * NUMHEAD, BF16,
                         bass.AP(tensor=gate_wT.tensor, offset=0,
                                 ap=[[NUMHEAD, 128], [128 * NUMHEAD, KC],
                                     [1, NUMHEAD]]))
                    wdma("cb", CB_COLS, F32R, cblob)
                    w["mb"] = []
                    for b in range(BPC):
                        t = wp.tile([128, 4], F32, tag=f"mb{b}", name=f"mb{b}")
                        nc.scalar.dma_start(out=t, in_=mblob[b])
                        w["mb"].append(t)
                    wdma("wq", KC * CHID, F32R, wsrc(q_wT))
                    wdma("wk", KC * CHID, F32R, wsrc(k_wT))
                    if USE_DR:
                        t = wp.tile([64, 2 * 128], F8E4, tag="id8", name="id8")
                        nc.scalar.dma_start(out=t, in_=id8in)
                        w["id8"] = t.rearrange("p (two m) -> p two m", two=2)
                        t = wp.tile([64, 2 * JC * L], F8E4, tag="rpt8", name="rpt8")
                        nc.scalar.dma_start(out=t, in_=rpt8in)
                        rt = t.rearrange("p (jc two l) -> p jc two l",
                                         two=2, l=L)
                        w["rpt8"] = [rt[:, jc] for jc in range(JC)]
                    else:
                        t = wp.tile([128, 128], BF16, tag="i128b", name="i128b")
                        nc.scalar.dma_start(out=t, in_=i128b)
                        w["id8"] = t
                        t = wp.tile([128, JC * L], BF16, tag="rptw", name="rptw")
                        nc.scalar.dma_start(out=t, in_=rptin)
                        w["rpt8"] = [t[:, jc * L:(jc + 1) * L]
                                     for jc in range(JC)]
                    wdma("wv", KC * CHID, F32R, wsrc(v_wT))
                    wdma("wd", KC * CHID, F32R, wsrc(dense_wT))
                    emit_inp(1, ab_engine=nc.sync)
                    first[0] = False
                else:
                    emit_inp(0)
                    emit_inp(1)
                w = weights
                cb, b8 = w["cb"], w["b8"]

                for b in range(BPC):
                    txa, txo, tab = binp[b]
                    xs = lambda t, kc: t[:, kc * L:(kc + 1) * L]

                    rn = a2.tile([128, 1], F32, tag="rn", name="rn")
                    nc.vector.reciprocal(rn, w["mb"][b][:, 3:4])

                    # ---- x0 = x + vres*xorg ; x1 = x + qko*xorg + qkp*abspos ----
                    tx0 = a1.tile([128, KC * L], F32R, tag="x0", name="x0")
                    tx1 = a1.tile([128, KC * L], F32R, tag="x1", name="x1")
                    ttm = a1.tile([128, KC * L], F32, tag="tt", name="tt")
                    tmv = a1.tile([128, KC * L], F32, tag="tmv", name="tmv")

                    def bcast1(col):
                        sv = cb[:, col:col + 1].bitcast(F32)
                        return bass.AP(tensor=sv.tensor, offset=sv.offset,
                                       ap=[sv.ap[0], [0, L]])

                    # x1 first (gates q/k projections): DVE fused ops
                    for kc in range(KC):
                        nc.vector.scalar_tensor_tensor(
                            xs(ttm, kc), xs(txo, kc),
                            cb[:, CB_QKO + kc:CB_QKO + kc + 1].bitcast(F32),
                            xs(txa, kc), OP.mult, OP.add)
                        nc.vector.scalar_tensor_tensor(
                            xs(tx1, kc), xs(tab, kc),
                            cb[:, CB_QKP + kc:CB_QKP + kc + 1].bitcast(F32),
                            xs(ttm, kc), OP.mult, OP.add)
                    # x0 (feeds vT later): Pool, off the critical path
                    for kc in range(KC):
                        nc.gpsimd.tensor_tensor(out=xs(tmv, kc), in0=xs(txo, kc),
                                                in1=bcast1(CB_VRES + kc), op=OP.mult)
                        nc.gpsimd.tensor_tensor(out=xs(tx0, kc), in0=xs(tmv, kc),
                                                in1=xs(txa, kc), op=OP.add)

                    # ---- gate projection (PE) ----
                    g_ps = psb.tile([NUMHEAD, L], F32, tag="big", name="g_ps")
                    for kc in range(KC):
                        nc.tensor.matmul(
                            g_ps, w["wg"][:, kc * NUMHEAD:(kc + 1) * NUMHEAD],
                            xs(txa, kc), start=(kc == 0), stop=(kc == KC - 1))
                    tgate = a2.tile([NUMHEAD, L], F32R, tag="gate", name="gate")
                    nc.scalar.activation(tgate, g_ps, AF.Identity,
                                         bias=b8[:, B8_GB:B8_GB + 1].bitcast(F32))

                    # ---- q/k projections (PE busy while gate transposes) ----
                    def proj(wall, rhs_all, nm):
                        dst = a2.tile([128, OC * L], F32R, tag=nm, name=nm)
                        for o in range(OC):
                            ps = psb.tile([128, L], F32, tag="big",
                                          name=f"{nm}p{o}")
                            for kc in range(KC):
                                nc.tensor.matmul(
                                    ps,
                                    wall[:, kc * CHID + o * 128:
                                         kc * CHID + o * 128 + 128],
                                    xs(rhs_all, kc),
                                    start=(kc == 0), stop=(kc == KC - 1))
                            if o < 2:
                                nc.scalar.copy(dst[:, o * L:(o + 1) * L], ps)
                            else:
                                nc.vector.tensor_copy(dst[:, o * L:(o + 1) * L], ps)
                        return dst

                    tgm = []

                    def emit_gt():
                        for jc in range(JC):
                            gt_ps = psb.tile([128, NUMHEAD], F32, tag="big",
                                             name=f"gt{jc}")
                            nc.tensor.matmul(gt_ps,
                                             tgate[:, jc * 128:(jc + 1) * 128],
                                             b8[:, B8_I8:B8_I8 + NUMHEAD],
                                             start=True, stop=True)
                            gm = a2.tile([128, NUMHEAD], F32, tag=f"gm{jc}",
                                         name=f"gm{jc}")
                            nc.vector.tensor_scalar(gm, gt_ps,
                                                    w["mb"][b][:, jc:jc + 1],
                                                    rn[:, 0:1], OP.add, OP.mult)
                            tgm.append(gm)

                    tq = a2.tile([128, OC * L], F32R, tag="q", name="q")
                    for o in range(OC):
                        ps = psb.tile([128, L], F32, tag="big", name=f"qp{o}")
                        for kc in range(KC):
                            nc.tensor.matmul(
                                ps,
                                w["wq"][:, kc * CHID + o * 128:
                                        kc * CHID + o * 128 + 128],
                                xs(tx1, kc),
                                start=(kc == 0), stop=(kc == KC - 1))
                        if o < 2:
                            nc.scalar.copy(tq[:, o * L:(o + 1) * L], ps)
                        else:
                            nc.vector.tensor_copy(tq[:, o * L:(o + 1) * L], ps)
                    emit_gt()

                    tk = proj(w["wk"], tx1, "k")

                    # ---- vT = x0^T @ v_wT ; per-head 65-col blocks, col 64 = ones ----
                    VW = DIMHEAD + 1
                    tvt = a2.tile([128, JC * NUMHEAD * VW], F32R, tag="vt", name="vt")

                    for lc in range(JC):
                        ps = psb.tile([128, CHID], F32, tag="big", name=f"vtp{lc}")
                        for kc in range(KC):
                            nc.tensor.matmul(
                                ps, tx0[:, kc * L + lc * 128:kc * L + lc * 128 + 128],
                                w["wv"][:, kc * CHID:(kc + 1) * CHID],
                                start=(kc == 0), stop=(kc == KC - 1))
                        blk = tvt[:, lc * NUMHEAD * VW:(lc + 1) * NUMHEAD * VW]
                        blk = blk.rearrange("p (h c) -> p h c", c=VW)
                        nc.vector.tensor_copy(blk[:, :, 0:DIMHEAD], ps)
                        nc.gpsimd.tensor_copy(blk[:, :, DIMHEAD:VW],
                                              cb[:, CB_ONES:CB_ONES + NUMHEAD])

                    # ---- attention (emission software-pipelined over heads) ----
                    tP_heads = {}
                    tatt = a1.tile([128, OC * L], F32R, tag="att", name="att")

                    def emit_qk(h):
                        hp, off = h // 2, 64 * (h % 2)
                        tP = []
                        sps = []
                        for jc in range(JC):
                            s_ps = pss.tile([128, L], F32, tag="s", name=f"s{h}{jc}")
                            nc.tensor.matmul(s_ps, w["id8"], w["rpt8"][jc],
                                             start=True, stop=False,
                                             perf_mode=(DR if USE_DR else None),
                                             skip_group_check=True)
                            sps.append(s_ps)
                        for jc in range(JC):
                            s_ps = sps[jc]
                            nc.tensor.matmul(
                                s_ps,
                                tk[off:off + 64,
                                   hp * L + jc * 128:hp * L + jc * 128 + 128],
                                tq[off:off + 64, hp * L:(hp + 1) * L],
                                start=False, stop=True, skip_group_check=True)
                            P = ppool.tile([128, L], F32R, tag="P", name=f"P{h}{jc}")
                            nc.scalar.activation(P, s_ps, AF.Exp,
                                                 bias=tgm[jc][:, h:h + 1],
                                                 scale=rn[:, 0:1])
                            tP.append(P)
                        tP_heads[h] = tP

                    # per-pair state: rcp tiles and rb psum tiles
                    pair_state = {}

                    def emit_av(h):
                        tP = tP_heads.pop(h)
                        hp, off = h // 2, 64 * (h % 2)
                        even = (h % 2 == 0)
                        o_ps = pso.tile([VW, L], F32, tag="o", name=f"o{h}")
                        for jc in range(JC):
                            nc.tensor.matmul(
                                o_ps,
                                tvt[:, jc * NUMHEAD * VW + VW * h:
                                    jc * NUMHEAD * VW + VW * h + VW],
                                tP[jc], start=(jc == 0), stop=(jc == JC - 1),
                                skip_group_check=True)
                        rcp = a2.tile([65, L], F32R, tag=f"rcp{h % 2}",
                                      name=f"rcp{h}")
                        with nc.allow_low_precision(reason="fp32r softmax recip"):
                            nc.vector.reciprocal(rcp[64:65, :], o_ps[64:65, :])
                        # broadcast 1/den to this head's 64 tatt rows
                        rb_ps = psb.tile([64, L], F32, tag="big", name=f"rb{h}")
                        nc.tensor.matmul(
                            rb_ps, cb[64:65, CB_ONES:CB_ONES + 64],
                            rcp[64:65, :],
                            start=True, stop=True, skip_group_check=True)
                        # evacuate v-rows to SBUF
                        oU = a1.tile([64, L], F32, tag=f"oU{h % 2}", name=f"oU{h}")
                        if h >= NUMHEAD - 2:
                            nc.scalar.copy(oU, o_ps[0:64, :])
                        else:
                            nc.vector.tensor_copy(oU, o_ps[0:64, :])
                        nc.vector.tensor_tensor(
                            out=tatt[off:off + 64, hp * L:(hp + 1) * L],
                            in0=oU, in1=rb_ps, op=OP.mult)

                    emit_qk(0)
                    emit_qk(1)
                    tout = a2.tile([128, OC * L], F32, tag="outt", name="outt")
                    d_ps = {}

                    def emit_dense_partial(o, kcs, stop):
                        if o not in d_ps:
                            d_ps[o] = psb.tile([128, L], F32, tag="big",
                                               name=f"dp{o}")
                        for kc in kcs:
                            nc.tensor.matmul(
                                d_ps[o],
                                w["wd"][:, kc * CIO + o * 128:kc * CIO + o * 128 + 128],
                                tatt[:, kc * L:(kc + 1) * L],
                                start=(kc == 0), stop=(stop and kc == kcs[-1]),
                                skip_group_check=True)
                        if stop:
                            nc.scalar.activation(
                                tout[:, o * L:(o + 1) * L], d_ps.pop(o), AF.Identity,
                                bias=cb[:, CB_DB + o:CB_DB + o + 1].bitcast(F32))

                    for h in range(NUMHEAD):
                        if h + 2 < NUMHEAD:
                            emit_qk(h + 2)
                        emit_av(h)
                        if h == NUMHEAD - 2:
                            for o in (0, 1):
                                emit_dense_partial(o, [0, 1, 2], stop=False)
                    for o in (0, 1):
                        emit_dense_partial(o, [3], stop=True)
                    for o in (2, 3):
                        emit_dense_partial(o, [0, 1, 2, 3], stop=True)
                    dst = bass.AP(tensor=out.tensor, offset=b * CIO * L,
                                  ap=[[L, 128], [128 * L, OC], [1, L]])
                    nc.sync.dma_start(out=dst, in_=tout)

    nc.compile()
    return nc


_CACHE = {}


def _get_nc(iters=1):
    if iters not in _CACHE:
        _CACHE[iters] = build_nc(iters)
    return _CACHE[iters]


def _f8(a):
    import ml_dtypes
    return np.asarray(a, dtype=ml_dtypes.float8_e4m3)


def _bf16(a):
    import ml_dtypes
    return np.asarray(a, dtype=ml_dtypes.bfloat16)


def _make_rpt8(relpos):
    # rpt[j, i] = relpos[clip(384 + j - i, 0, 766)], j = global key pos
    j = np.arange(L)[:, None]
    i = np.arange(L)[None, :]
    idx = np.clip(MAXEMBED + j - i, 0, 2 * MAXEMBED - 2)
    rp = relpos[idx]                                  # [j, i]
    # DoubleRow pack with j = 128*jc + 2p + r:
    # rpt8[p, jc, r, i] = rp[128*jc + 2p + r, i] -> flat [64, JC*2*L],
    # each jc block contiguous [2, L]
    rp4 = rp.reshape(JC, 64, 2, L).transpose(1, 0, 2, 3)
    return np.ascontiguousarray(rp4.reshape(64, 2 * JC * L))


def _host_prep(inputs):
    f32 = lambda a: np.ascontiguousarray(np.asarray(a), dtype=np.float32)
    x, xorg, abspos = f32(inputs["x"]), f32(inputs["xorg"]), f32(inputs["abspos"])
    mask, norm = f32(inputs["mask"]), f32(inputs["norm"])
    relpos = f32(inputs["relpos"])

    def expand_res(r):
        e = np.repeat(f32(r).reshape(-1), DIMGROUP)          # [512]
        return np.ascontiguousarray(e.reshape(KC, 128).T)    # [128, KC]

    cblob = np.zeros((128, CB_COLS), np.float32)
    cblob[:, CB_VRES:CB_VRES + KC] = expand_res(inputs["vorg_res"])
    cblob[:, CB_QKO:CB_QKO + KC] = expand_res(inputs["qkorg_res"])
    cblob[:, CB_QKP:CB_QKP + KC] = expand_res(inputs["qkpos_res"])
    cblob[:, CB_DB:CB_DB + KC] = np.ascontiguousarray(
        f32(inputs["dense_b"]).reshape(OC, 128).T)
    cblob[:, CB_ONES:CB_ONES + 64] = 1.0

    blob8 = np.zeros((NUMHEAD, B8_COLS), np.float32)
    blob8[:, B8_I8:B8_I8 + NUMHEAD] = np.eye(NUMHEAD, dtype=np.float32)
    blob8[:, B8_GB] = f32(inputs["gate_b"])

    # DoubleRow identity: id8[p, r, m] = 1 iff m == 2p + r
    id8 = np.zeros((64, 2, 128), np.float32)
    p = np.arange(64)
    id8[p, 0, 2 * p] = 1.0
    id8[p, 1, 2 * p + 1] = 1.0

    shared = {
        "gate_wT": _bf16(np.ascontiguousarray(f32(inputs["gate_w"]).T)),
        "q_wT": np.ascontiguousarray(f32(inputs["q_w"]).T),
        "k_wT": np.ascontiguousarray(f32(inputs["k_w"]).T),
        "v_wT": np.ascontiguousarray(f32(inputs["v_w"]).T),
        "dense_wT": np.ascontiguousarray(f32(inputs["dense_w"]).T),
        "cblob": cblob,
        "blob8": blob8,
    }
    if USE_DR:
        shared["rpt8in"] = _f8(_make_rpt8(relpos))
        shared["id8in"] = _f8(id8.reshape(64, 256))
    else:
        import ml_dtypes
        j = np.arange(L)[:, None]
        i = np.arange(L)[None, :]
        idx = np.clip(MAXEMBED + j - i, 0, 2 * MAXEMBED - 2)
        rp = relpos[idx].reshape(JC, 128, L).transpose(1, 0, 2)
        shared["rptin"] = np.asarray(
            np.ascontiguousarray(rp.reshape(128, JC * L)), ml_dtypes.bfloat16)
        shared["i128b"] = np.asarray(np.eye(128, dtype=np.float32),
                                     ml_dtypes.bfloat16)
    # mblob: [N, 128, 4] = [maskT(3) | norm(1)]
    mblob = np.zeros((N, 128, 4), np.float32)
    mblob[:, :, 0:3] = mask.reshape(N, JC, 128).transpose(0, 2, 1)
    mblob[:, :, 3] = norm[:, None]
    in_maps = []
    for c in range(NCORES):
        sl = slice(BPC * c, BPC * (c + 1))
        m = dict(shared)
        m["x"] = _bf16(x[sl])
        m["xorg"] = _bf16(xorg[sl])
        m["abspos"] = _bf16(abspos[sl])
        m["mblob"] = mblob[sl]
        in_maps.append(m)
    return in_maps


def run_on_hw(inputs, iters=1):
    nc = _get_nc(iters)
    in_maps = _host_prep(inputs)
    res = run_bass_kernel_spmd(nc, in_maps, list(range(NCORES)))
    return np.concatenate([res.results[c]["out"] for c in range(NCORES)], axis=0)


def kernel(**inputs) -> np.ndarray:
    return run_on_hw(inputs, iters=1)


# revision 60
# speedup vs baseline: 1.0802x; 1.0802x over previous
import sys
sys.path.insert(0, "/opt/trn_rl_repo")
import numpy as np
import concourse.bass as bass
import concourse.bacc as bacc
import concourse.tile as tile
from concourse import mybir
from concourse.bass_utils import run_bass_kernel_spmd
from concourse import bass_isa

F32 = mybir.dt.float32
F32R = mybir.dt.float32r
BF16 = mybir.dt.bfloat16
F8E4 = mybir.dt.float8e4
AF = mybir.ActivationFunctionType
OP = mybir.AluOpType
DR = mybir.MatmulPerfMode.DoubleRow

USE_DR = False          # fp8 DoubleRow for the relpos-add matmul
N, CIO, L = 16, 512, 384
DIMHEAD, NUMHEAD, MAXEMBED, DIMGROUP = 64, 8, 384, 8
CHID = DIMHEAD * NUMHEAD
NCORES = 8
BPC = N // NCORES      # batches per core
KC = CIO // 128        # contraction chunks
OC = CHID // 128       # output-channel chunks
JC = L // 128          # key-position chunks

# constant-blob column layout (128-partition blob, fp32r-typed)
CB_VRES = 0            # [128,4]
CB_QKO = 4             # [128,4]
CB_QKP = 8             # [128,4]
CB_DB = 12             # [128,4] dense bias
CB_ONES = 16           # [128,64] ones (vT one-columns + rb broadcast lhsT)
CB_COLS = 80
# 8-partition blob
B8_I8 = 0              # [8,8]
B8_GB = 8              # [8,1] gate bias
B8_COLS = 9


def build_nc(iters=1):
    nc = bacc.Bacc("TRN2", target_bir_lowering=False, debug=False)

    def inp(name, shape, dt):
        return nc.dram_tensor(name, shape, dt, kind="ExternalInput").ap()

    x = inp("x", [BPC, CIO, L], BF16)
    xorg = inp("xorg", [BPC, CIO, L], BF16)
    abspos = inp("abspos", [BPC, CIO, L], BF16)
    mblob = inp("mblob", [BPC, 128, 4], F32)     # [maskT(3) | norm(1)]
    if USE_DR:
        rpt8in = inp("rpt8in", [64, 2 * JC * L], F8E4)
        id8in = inp("id8in", [64, 2 * 128], F8E4)
    else:
        rptin = inp("rptin", [128, JC * L], BF16)
        i128b = inp("i128b", [128, 128], BF16)
    gate_wT = inp("gate_wT", [CIO, NUMHEAD], BF16)
    q_wT = inp("q_wT", [CIO, CHID], F32R)
    k_wT = inp("k_wT", [CIO, CHID], F32R)
    v_wT = inp("v_wT", [CIO, CHID], F32R)
    dense_wT = inp("dense_wT", [CHID, CIO], F32R)
    cblob = inp("cblob", [128, CB_COLS], F32R)
    blob8 = inp("blob8", [NUMHEAD, B8_COLS], F32R)
    out = nc.dram_tensor("out", [BPC, CIO, L], F32, kind="ExternalOutput").ap()

    def chunked_src(t, b):
        # [CIO, L] DRAM slice viewed as [p(128), kc(4), l(384)]
        return bass.AP(tensor=t.tensor, offset=b * CIO * L,
                       ap=[[L, 128], [128 * L, KC], [1, L]])

    def wsrc(t):
        # [CIO, CHID] viewed as [p(128), kc(4), c(CHID)]
        return bass.AP(tensor=t.tensor, offset=0,
                       ap=[[CHID, 128], [128 * CHID, KC], [1, CHID]])

    with tile.TileContext(nc) as tc:
        with tc.tile_pool(name="wts", bufs=1) as wp, \
             tc.tile_pool(name="a2", bufs=2) as a2, \
             tc.tile_pool(name="a1", bufs=1) as a1, \
             tc.tile_pool(name="pp", bufs=6) as ppool, \
             tc.tile_pool(name="ps_big", bufs=3, space="PSUM") as psb, \
             tc.tile_pool(name="ps_s", bufs=3, space="PSUM") as pss, \
             tc.tile_pool(name="ps_o", bufs=2, space="PSUM") as pso:

            first = [True]
            weights = {}

            for it in range(iters):
                binp = []

                def emit_inp(b, ab_engine=None):
                    txa = a1.tile([128, KC * L], BF16, tag=f"xa{b}", name=f"xa{b}")
                    nc.sync.dma_start(out=txa, in_=chunked_src(x, b))
                    txo = a1.tile([128, KC * L], BF16, tag=f"xo{b}", name=f"xo{b}")
                    nc.sync.dma_start(out=txo, in_=chunked_src(xorg, b))
                    tab = a1.tile([128, KC * L], BF16, tag=f"ab{b}", name=f"ab{b}")
                    (ab_engine or nc.gpsimd).dma_start(out=tab,
                                                       in_=chunked_src(abspos, b))
                    binp.append((txa, txo, tab))

                def wdma(nm, shape, dt, srcap):
                    t = wp.tile([128, shape], dt, tag=nm, name=nm)
                    nc.scalar.dma_start(out=t, in_=srcap)
                    weights[nm] = t
                    return t

                if first[0]:
                    w = weights
                    emit_inp(0)
                    t = wp.tile([NUMHEAD, B8_COLS], F32R, tag="b8", name="b8")
                    nc.scalar.dma_start(out=t, in_=blob8)
                    w["b8"] = t
                    wdma("wg", KC * NUMHEAD, BF16,
                         bass.AP(tensor=gate_wT.tensor, offset=0,
                                 ap=[[NUMHEAD, 128], [128 * NUMHEAD, KC],
                                     [1, NUMHEAD]]))
                    wdma("cb", CB_COLS, F32R, cblob)
                    w["mb"] = []
                    for b in range(BPC):
                        t = wp.tile([128, 4], F32, tag=f"mb{b}", name=f"mb{b}")
                        nc.scalar.dma_start(out=t, in_=mblob[b])
                        w["mb"].append(t)
                    wdma("wq", KC * CHID, F32R, wsrc(q_wT))
                    wdma("wk", KC * CHID, F32R, wsrc(k_wT))
                    if USE_DR:
                        t = wp.tile([64, 2 * 128], F8E4, tag="id8", name="id8")
                        nc.scalar.dma_start(out=t, in_=id8in)
                        w["id8"] = t.rearrange("p (two m) -> p two m", two=2)
                        t = wp.tile([64, 2 * JC * L], F8E4, tag="rpt8", name="rpt8")
                        nc.scalar.dma_start(out=t, in_=rpt8in)
                        rt = t.rearrange("p (jc two l) -> p jc two l",
                                         two=2, l=L)
                        w["rpt8"] = [rt[:, jc] for jc in range(JC)]
                    else:
                        t = wp.tile([128, 128], BF16, tag="i128b", name="i128b")
                        nc.scalar.dma_start(out=t, in_=i128b)
                        w["id8"] = t
                        t = wp.tile([128, JC * L], BF16, tag="rptw", name="rptw")
                        nc.scalar.dma_start(out=t, in_=rptin)
                        w["rpt8"] = [t[:, jc * L:(jc + 1) * L]
                                     for jc in range(JC)]
                    wdma("wv", KC * CHID, F32R, wsrc(v_wT))
                    wdma("wd", KC * CHID, F32R, wsrc(dense_wT))
                    emit_inp(1, ab_engine=nc.sync)
                    first[0] = False
                else:
                    emit_inp(0)
                    emit_inp(1)
                w = weights
                cb, b8 = w["cb"], w["b8"]

                for b in range(BPC):
                    txa, txo, tab = binp[b]
                    xs = lambda t, kc: t[:, kc * L:(kc + 1) * L]

                    rn = a2.tile([128, 1], F32, tag="rn", name="rn")
                    nc.vector.reciprocal(rn, w["mb"][b][:, 3:4])

                    # ---- x0 = x + vres*xorg ; x1 = x + qko*xorg + qkp*abspos ----
                    tx0 = a1.tile([128, KC * L], F32R, tag="x0", name="x0")
                    tx1 = a1.tile([128, KC * L], F32R, tag="x1", name="x1")
                    ttm = a1.tile([128, KC * L], F32, tag="tt", name="tt")
                    tmv = a1.tile([128, KC * L], F32, tag="tmv", name="tmv")

                    def bcast1(col):
                        sv = cb[:, col:col + 1].bitcast(F32)
                        return bass.AP(tensor=sv.tensor, offset=sv.offset,
                                       ap=[sv.ap[0], [0, L]])

                    # x1 first (gates q/k projections): DVE fused ops
                    for kc in range(KC):
                        nc.vector.scalar_tensor_tensor(
                            xs(ttm, kc), xs(txo, kc),
                            cb[:, CB_QKO + kc:CB_QKO + kc + 1].bitcast(F32),
                            xs(txa, kc), OP.mult, OP.add)
                        nc.vector.scalar_tensor_tensor(
                            xs(tx1, kc), xs(tab, kc),
                            cb[:, CB_QKP + kc:CB_QKP + kc + 1].bitcast(F32),
                            xs(ttm, kc), OP.mult, OP.add)
                    # x0 (feeds vT later): Pool, off the critical path
                    for kc in range(KC):
                        nc.gpsimd.tensor_tensor(out=xs(tmv, kc), in0=xs(txo, kc),
                                                in1=bcast1(CB_VRES + kc), op=OP.mult)
                        nc.gpsimd.tensor_tensor(out=xs(tx0, kc), in0=xs(tmv, kc),
                                                in1=xs(txa, kc), op=OP.add)

                    # ---- gate projection (PE) ----
                    g_ps = psb.tile([NUMHEAD, L], F32, tag="big", name="g_ps")
                    for kc in range(KC):
                        nc.tensor.matmul(
                            g_ps, w["wg"][:, kc * NUMHEAD:(kc + 1) * NUMHEAD],
                            xs(txa, kc), start=(kc == 0), stop=(kc == KC - 1))
                    tgate = a2.tile([NUMHEAD, L], F32R, tag="gate", name="gate")
                    nc.scalar.activation(tgate, g_ps, AF.Identity,
                                         bias=b8[:, B8_GB:B8_GB + 1].bitcast(F32))

                    # ---- q/k projections (PE busy while gate transposes) ----
                    def proj(wall, rhs_all, nm):
                        dst = a2.tile([128, OC * L], F32R, tag=nm, name=nm)
                        for o in range(OC):
                            ps = psb.tile([128, L], F32, tag="big",
                                          name=f"{nm}p{o}")
                            for kc in range(KC):
                                nc.tensor.matmul(
                                    ps,
                                    wall[:, kc * CHID + o * 128:
                                         kc * CHID + o * 128 + 128],
                                    xs(rhs_all, kc),
                                    start=(kc == 0), stop=(kc == KC - 1))
                            if o < 2:
                                nc.scalar.copy(dst[:, o * L:(o + 1) * L], ps)
                            else:
                                nc.vector.tensor_copy(dst[:, o * L:(o + 1) * L], ps)
                        return dst

                    tgm = []

                    def emit_gt():
                        # all 3 j-chunks into one PSUM tile, one fused DVE op
                        maskrn = a2.tile([128, JC], F32, tag="mrn", name="mrn")
                        nc.vector.tensor_scalar_mul(maskrn, w["mb"][b][:, 0:JC],
                                                    rn[:, 0:1])
                        gt_ps = psb.tile([128, JC * NUMHEAD], F32, tag="big",
                                         name="gt")
                        for jc in range(JC):
                            nc.tensor.matmul(
                                gt_ps[:, jc * NUMHEAD:(jc + 1) * NUMHEAD],
                                tgate[:, jc * 128:(jc + 1) * 128],
                                b8[:, B8_I8:B8_I8 + NUMHEAD],
                                start=True, stop=True, skip_group_check=True)
                        gm = a2.tile([128, JC * NUMHEAD], F32, tag="gm",
                                     name="gm")
                        mrn_b = bass.AP(tensor=maskrn.tensor, offset=maskrn.offset,
                                        ap=[maskrn.ap[0], [1, JC], [0, NUMHEAD]])
                        nc.vector.scalar_tensor_tensor(
                            gm.rearrange("p (jc h) -> p jc h", h=NUMHEAD),
                            gt_ps.rearrange("p (jc h) -> p jc h", h=NUMHEAD),
                            rn[:, 0:1], mrn_b, OP.mult, OP.add)
                        for jc in range(JC):
                            tgm.append(gm[:, jc * NUMHEAD:(jc + 1) * NUMHEAD])

                    # ---- vT = x0^T @ v_wT ; before q/k so PE has work while
                    # the DVE finishes x1 ----
                    VW = DIMHEAD + 1
                    tvt = a2.tile([128, JC * NUMHEAD * VW], F32R, tag="vt", name="vt")

                    for lc in range(JC):
                        ps = psb.tile([128, CHID], F32, tag="big", name=f"vtp{lc}")
                        for kc in range(KC):
                            nc.tensor.matmul(
                                ps, tx0[:, kc * L + lc * 128:kc * L + lc * 128 + 128],
                                w["wv"][:, kc * CHID:(kc + 1) * CHID],
                                start=(kc == 0), stop=(kc == KC - 1))
                        blk = tvt[:, lc * NUMHEAD * VW:(lc + 1) * NUMHEAD * VW]
                        blk = blk.rearrange("p (h c) -> p h c", c=VW)
                        nc.vector.tensor_copy(blk[:, :, 0:DIMHEAD], ps)
                        nc.gpsimd.tensor_copy(blk[:, :, DIMHEAD:VW],
                                              cb[:, CB_ONES:CB_ONES + NUMHEAD])

                    tq = a2.tile([128, OC * L], F32R, tag="q", name="q")
                    for o in range(OC):
                        ps = psb.tile([128, L], F32, tag="big", name=f"qp{o}")
                        for kc in range(KC):
                            nc.tensor.matmul(
                                ps,
                                w["wq"][:, kc * CHID + o * 128:
                                        kc * CHID + o * 128 + 128],
                                xs(tx1, kc),
                                start=(kc == 0), stop=(kc == KC - 1))
                        if o < 2:
                            nc.scalar.copy(tq[:, o * L:(o + 1) * L], ps)
                        else:
                            nc.vector.tensor_copy(tq[:, o * L:(o + 1) * L], ps)
                    emit_gt()

                    tk = proj(w["wk"], tx1, "k")

                    # ---- attention (emission software-pipelined over heads) ----
                    tP_heads = {}
                    tatt = a1.tile([128, OC * L], F32R, tag="att", name="att")

                    def emit_qk(h):
                        hp, off = h // 2, 64 * (h % 2)
                        tP = []
                        sps = []
                        for jc in range(JC):
                            s_ps = pss.tile([128, L], F32, tag="s", name=f"s{h}{jc}")
                            nc.tensor.matmul(s_ps, w["id8"], w["rpt8"][jc],
                                             start=True, stop=False,
                                             perf_mode=(DR if USE_DR else None),
                                             skip_group_check=True)
                            sps.append(s_ps)
                        for jc in range(JC):
                            s_ps = sps[jc]
                            nc.tensor.matmul(
                                s_ps,
                                tk[off:off + 64,
                                   hp * L + jc * 128:hp * L + jc * 128 + 128],
                                tq[off:off + 64, hp * L:(hp + 1) * L],
                                start=False, stop=True, skip_group_check=True)
                            P = ppool.tile([128, L], F32R, tag="P", name=f"P{h}{jc}")
                            nc.scalar.activation(P, s_ps, AF.Exp,
                                                 bias=tgm[jc][:, h:h + 1],
                                                 scale=rn[:, 0:1])
                            tP.append(P)
                        tP_heads[h] = tP

                    # per-pair state: rcp tiles and rb psum tiles
                    pair_state = {}

                    def emit_av(h):
                        tP = tP_heads.pop(h)
                        hp, off = h // 2, 64 * (h % 2)
                        even = (h % 2 == 0)
                        o_ps = pso.tile([VW, L], F32, tag="o", name=f"o{h}")
                        for jc in range(JC):
                            nc.tensor.matmul(
                                o_ps,
                                tvt[:, jc * NUMHEAD * VW + VW * h:
                                    jc * NUMHEAD * VW + VW * h + VW],
                                tP[jc], start=(jc == 0), stop=(jc == JC - 1),
                                skip_group_check=True)
                        rcp = a2.tile([65, L], F32R, tag=f"rcp{h % 2}",
                                      name=f"rcp{h}")
                        with nc.allow_low_precision(reason="fp32r softmax recip"):
                            nc.vector.reciprocal(rcp[64:65, :], o_ps[64:65, :])
                        # broadcast 1/den to this head's 64 tatt rows
                        rb_ps = psb.tile([64, L], F32, tag="big", name=f"rb{h}")
                        nc.tensor.matmul(
                            rb_ps, cb[64:65, CB_ONES:CB_ONES + 64],
                            rcp[64:65, :],
                            start=True, stop=True, skip_group_check=True)
                        # evacuate v-rows to SBUF, then normalize
                        oU = a1.tile([64, L], F32, tag=f"oU{h % 2}", name=f"oU{h}")
                        if h >= NUMHEAD - 2:
                            nc.scalar.copy(oU, o_ps[0:64, :])
                        else:
                            nc.vector.tensor_copy(oU, o_ps[0:64, :])
                        nc.vector.tensor_tensor(
                            out=tatt[off:off + 64, hp * L:(hp + 1) * L],
                            in0=oU, in1=rb_ps, op=OP.mult)

                    emit_qk(0)
                    emit_qk(1)
                    tout = a2.tile([128, OC * L], F32, tag="outt", name="outt")
                    d_ps = {}

                    def emit_dense_partial(o, kcs, stop):
                        if o not in d_ps:
                            d_ps[o] = psb.tile([128, L], F32, tag="big",
                                               name=f"dp{o}")
                        for kc in kcs:
                            nc.tensor.matmul(
                                d_ps[o],
                                w["wd"][:, kc * CIO + o * 128:kc * CIO + o * 128 + 128],
                                tatt[:, kc * L:(kc + 1) * L],
                                start=(kc == 0), stop=(stop and kc == kcs[-1]),
                                skip_group_check=True)
                        if stop:
                            nc.scalar.activation(
                                tout[:, o * L:(o + 1) * L], d_ps.pop(o), AF.Identity,
                                bias=cb[:, CB_DB + o:CB_DB + o + 1].bitcast(F32))

                    for h in range(NUMHEAD):
                        if h + 2 < NUMHEAD:
                            emit_qk(h + 2)
                        emit_av(h)
                        if h == NUMHEAD - 2:
                            for o in (0, 1):
                                emit_dense_partial(o, [0, 1, 2], stop=False)
                    for o in (0, 1):
                        emit_dense_partial(o, [3], stop=True)
                    for o in (2, 3):
                        emit_dense_partial(o, [0, 1, 2, 3], stop=True)
                    dst = bass.AP(tensor=out.tensor, offset=b * CIO * L,
                                  ap=[[L, 128], [128 * L, OC], [1, L]])
                    nc.sync.dma_start(out=dst, in_=tout)

    nc.compile()
    return nc


_CACHE = {}


def _get_nc(iters=1):
    if iters not in _CACHE:
        _CACHE[iters] = build_nc(iters)
    return _CACHE[iters]


def _f8(a):
    import ml_dtypes
    return np.asarray(a, dtype=ml_dtypes.float8_e4m3)


def _bf16(a):
    import ml_dtypes
    return np.asarray(a, dtype=ml_dtypes.bfloat16)


def _make_rpt8(relpos):
    # rpt[j, i] = relpos[clip(384 + j - i, 0, 766)], j = global key pos
    j = np.arange(L)[:, None]
    i = np.arange(L)[None, :]
    idx = np.clip(MAXEMBED + j - i, 0, 2 * MAXEMBED - 2)
    rp = relpos[idx]                                  # [j, i]
    # DoubleRow pack with j = 128*jc + 2p + r:
    # rpt8[p, jc, r, i] = rp[128*jc + 2p + r, i] -> flat [64, JC*2*L],
    # each jc block contiguous [2, L]
    rp4 = rp.reshape(JC, 64, 2, L).transpose(1, 0, 2, 3)
    return np.ascontiguousarray(rp4.reshape(64, 2 * JC * L))


def _host_prep(inputs):
    f32 = lambda a: np.ascontiguousarray(np.asarray(a), dtype=np.float32)
    x, xorg, abspos = f32(inputs["x"]), f32(inputs["xorg"]), f32(inputs["abspos"])
    mask, norm = f32(inputs["mask"]), f32(inputs["norm"])
    relpos = f32(inputs["relpos"])

    def expand_res(r):
        e = np.repeat(f32(r).reshape(-1), DIMGROUP)          # [512]
        return np.ascontiguousarray(e.reshape(KC, 128).T)    # [128, KC]

    cblob = np.zeros((128, CB_COLS), np.float32)
    cblob[:, CB_VRES:CB_VRES + KC] = expand_res(inputs["vorg_res"])
    cblob[:, CB_QKO:CB_QKO + KC] = expand_res(inputs["qkorg_res"])
    cblob[:, CB_QKP:CB_QKP + KC] = expand_res(inputs["qkpos_res"])
    cblob[:, CB_DB:CB_DB + KC] = np.ascontiguousarray(
        f32(inputs["dense_b"]).reshape(OC, 128).T)
    cblob[:, CB_ONES:CB_ONES + 64] = 1.0

    blob8 = np.zeros((NUMHEAD, B8_COLS), np.float32)
    blob8[:, B8_I8:B8_I8 + NUMHEAD] = np.eye(NUMHEAD, dtype=np.float32)
    blob8[:, B8_GB] = f32(inputs["gate_b"])

    # DoubleRow identity: id8[p, r, m] = 1 iff m == 2p + r
    id8 = np.zeros((64, 2, 128), np.float32)
    p = np.arange(64)
    id8[p, 0, 2 * p] = 1.0
    id8[p, 1, 2 * p + 1] = 1.0

    shared = {
        "gate_wT": _bf16(np.ascontiguousarray(f32(inputs["gate_w"]).T)),
        "q_wT": np.ascontiguousarray(f32(inputs["q_w"]).T),
        "k_wT": np.ascontiguousarray(f32(inputs["k_w"]).T),
        "v_wT": np.ascontiguousarray(f32(inputs["v_w"]).T),
        "dense_wT": np.ascontiguousarray(f32(inputs["dense_w"]).T),
        "cblob": cblob,
        "blob8": blob8,
    }
    if USE_DR:
        shared["rpt8in"] = _f8(_make_rpt8(relpos))
        shared["id8in"] = _f8(id8.reshape(64, 256))
    else:
        import ml_dtypes
        j = np.arange(L)[:, None]
        i = np.arange(L)[None, :]
        idx = np.clip(MAXEMBED + j - i, 0, 2 * MAXEMBED - 2)
        rp = relpos[idx].reshape(JC, 128, L).transpose(1, 0, 2)
        shared["rptin"] = np.asarray(
            np.ascontiguousarray(rp.reshape(128, JC * L)), ml_dtypes.bfloat16)
        shared["i128b"] = np.asarray(np.eye(128, dtype=np.float32),
                                     ml_dtypes.bfloat16)
    # mblob: [N, 128, 4] = [maskT(3) | norm(1)]
    mblob = np.zeros((N, 128, 4), np.float32)
    mblob[:, :, 0:3] = mask.reshape(N, JC, 128).transpose(0, 2, 1)
    mblob[:, :, 3] = norm[:, None]
    in_maps = []
    for c in range(NCORES):
        sl = slice(BPC * c, BPC * (c + 1))
        m = dict(shared)
        m["x"] = _bf16(x[sl])
        m["xorg"] = _bf16(xorg[sl])
        m["abspos"] = _bf16(abspos[sl])
        m["mblob"] = mblob[sl]
        in_maps.append(m)
    return in_maps


def run_on_hw(inputs, iters=1):
    nc = _get_nc(iters)
    in_maps = _host_prep(inputs)
    res = run_bass_kernel_spmd(nc, in_maps, list(range(NCORES)))
    return np.concatenate([res.results[c]["out"] for c in range(NCORES)], axis=0)


def kernel(**inputs) -> np.ndarray:
    return run_on_hw(inputs, iters=1)
